# revision 1
# baseline (speedup 1.0000x reference)
# Cross-attention kernel for Trainium2, 8 NeuronCores.
#
# Sharding: data-parallel over (batch, query-half): core = 2*b + half handles
# batch b, queries [half*1024, (half+1)*1024). No collectives.
#
# On-device layout is feature-major ("transposed"): activations live as
# [feature, token]. The host pre-transposes inputs and post-transposes the
# output. Both layernorms are folded into the projections:
#   LN(x) @ W.T = (x*r) @ W'.T - (m*r) x S + bq, with W' = (W*g).T,
#   S[j] = sum_d W'[d,j], bq = W @ b, applied via a K=2 auxiliary matmul
#   with moving rows [m*r; 1].
# Attention runs with keys on partitions (logits transposed), so softmax
# denominators come from an all-ones column appended to V (M=65 matmul
# output row 64 = sum of exp). Max-subtraction is skipped: |logits/temp| < 3.
import os
import sys
import tempfile

# The neuron compile cache keys on the jax module hash, which does not cover
# the embedded Bass program — a stale NEFF can be silently reused. Use a
# fresh cache dir per process.
os.environ["NEURON_COMPILE_CACHE_URL"] = tempfile.mkdtemp(prefix="neff_cache_")
# The axon cassette (compile record/replay) fingerprints the module interface
# but not the embedded Bass program; salt it per process so edits always
# recompile instead of replaying a stale executable.
os.environ["AXON_CASSETTE_SALT"] = f"ca-{os.getpid()}-{os.urandom(4).hex()}"

for _p in ("/opt/trn_rl_repo",):
    if os.path.isdir(_p) and _p not in sys.path:
        sys.path.insert(0, _p)

import numpy as np
import ml_dtypes
from contextlib import ExitStack

import concourse.bass as bass
import concourse.tile as tile
from concourse import bacc, mybir
from concourse.bass_utils import run_bass_kernel_spmd

F32 = mybir.dt.float32
F32R = mybir.dt.float32r
BF16 = mybir.dt.bfloat16
AF = mybir.ActivationFunctionType

B, NQ, NK, D = 4, 2048, 2048, 512
H, DH = 8, 64
NQS = NQ // 2  # queries per core
TEMP = float(np.sqrt(512.0))
LN_EPS = 1e-5
N_CORES = 8

_CACHE = {}
# Interface salt: the remote executable cache fingerprints the module
# interface but not the embedded program; bump per kernel edit to force
# recompilation.
SALT = "v09"
SALT_N = 9


def _build_program():
    nc = bacc.Bacc("TRN2", target_bir_lowering=False, debug=False)

    def din(name, shape, dt=F32R):
        return nc.dram_tensor(f"{name}_{SALT}", shape, dt,
                              kind="ExternalInput").ap()

    qt_d = din("qt", [128, 4, NQS], BF16)
    kt_d = din("kt", [128, 4, NK], BF16)
    vt_d = din("vt", [128, 4, NK], BF16)
    wq_d = din("wq", [128, 4, D], BF16)
    wk_d = din("wk", [128, 4, D], BF16)
    wv_d = din("wv", [128, 4, D], BF16)
    wo_d = din("wo", [128, 4, D])
    aq_d = din("aq", [2, D], BF16)
    ak_d = din("ak", [2, D], BF16)
    ao_d = din("ao", [2, D])
    gb_d = din("gb", [128, 2, 4], F32)
    salt_d = din("salt", [1, 8 + SALT_N], F32)
    out_d = nc.dram_tensor(f"out_{SALT}", [128, 4, NQS], F32,
                           kind="ExternalOutput").ap()

    with tile.TileContext(nc) as tc, ExitStack() as top:
        persist = top.enter_context(tc.tile_pool(name="persist", bufs=1))
        qTs = persist.tile([128, 4, NQS], BF16)       # projected q, feature-major
        kTs = persist.tile([128, 4, NK], BF16)        # projected k
        vaug = persist.tile([128, 16, 8, 65], BF16)   # v natural + ones col per head
        oTs = persist.tile([128, 4, NQS], F32R)       # normalized attention out
        wo_sb = persist.tile([128, 4, D], F32R)
        ao_sb = persist.tile([2, D], F32R)
        gb_sb = persist.tile([128, 2, 4], F32)
        ones32 = persist.tile([128, NK], F32)
        onesr = persist.tile([128, 128], F32R)
        onesb = persist.tile([128, 128], BF16)
        ones_rb = persist.tile([1, NK], BF16)
        eps_t = persist.tile([128, 1], F32)

        nc.sync.dma_start(out=wo_sb, in_=wo_d)
        nc.sync.dma_start(out=ao_sb, in_=ao_d)
        nc.sync.dma_start(out=gb_sb, in_=gb_d)
        nc.vector.memset(ones32, 1.0)
        nc.sync.dma_start(out=eps_t, in_=salt_d[0:1, 0:1].to_broadcast([128, 1]))
        nc.vector.tensor_copy(onesr, ones32[:, 0:128])
        nc.vector.memset(onesb, 1.0)
        nc.vector.memset(ones_rb, 1.0)
        ones_r = persist.tile([1, NK], F32R)
        nc.vector.tensor_copy(ones_r, ones32[0:1, :])
        # ones columns of vaug (slot 64 of each head's lhsT)
        nc.vector.memset(vaug[:, :, :, 64], 1.0)

        def ln_stats_prescale(xin, nchunks, st_ps, work, aux):
            """Column LN stats of xin [128, 4, nchunks*512]; prescales xin by
            r in place; fills aux [2, nchunks*512] rows with [m*r; 1]."""
            bf = xin.dtype == BF16
            o_mm = onesb if bf else onesr
            nc.sync.dma_start(
                out=aux[1:2, :],
                in_=(ones_rb if bf else ones_r)[0:1, 0 : nchunks * 512])
            for n2 in range(nchunks):
                ns = slice(512 * n2, 512 * n2 + 512)
                ps_sum = st_ps.tile([128, 512], F32, name="ps_sum")
                ps_ssq = st_ps.tile([128, 512], F32, name="ps_ssq")
                for jc in range(4):
                    nc.tensor.matmul(ps_sum, o_mm, xin[:, jc, ns],
                                     start=(jc == 0), stop=(jc == 3))
                for jc in range(4):
                    sq = work.tile([128, 512], BF16 if bf else F32R,
                                   name="sq", bufs=3)
                    with nc.allow_low_precision("f32r keeps fp32 storage"):
                        nc.vector.tensor_mul(sq, xin[:, jc, ns],
                                             xin[:, jc, ns])
                    nc.tensor.matmul(ps_ssq, o_mm, sq,
                                     start=(jc == 0), stop=(jc == 3))
                # replicated stats rows: m = sum/512 ; var = (ssq - sum*m)/512
                m_b = work.tile([128, 512], F32, name="m_b", bufs=2)
                nc.scalar.mul(m_b, ps_sum, 1.0 / 512.0)
                t2 = work.tile([128, 512], F32, name="t2", bufs=2)
                nc.vector.tensor_mul(t2, m_b, ps_sum)
                dv = work.tile([128, 512], F32, name="dv", bufs=2)
                nc.vector.tensor_sub(dv, ps_ssq, t2)
                std = work.tile([128, 512], F32, name="std", bufs=2)
                nc.scalar.activation(std, dv, AF.Sqrt, bias=eps_t,
                                     scale=1.0 / 512.0)
                r_b = work.tile([128, 512], F32, name="r_b", bufs=2)
                nc.vector.reciprocal(r_b, std)
                mr_b = work.tile([128, 512], F32R, name="mr_b", bufs=2)
                with nc.allow_low_precision("f32r keeps fp32 storage"):
                    nc.vector.tensor_mul(mr_b, m_b, r_b)
                nc.vector.tensor_copy(aux[0:1, ns], mr_b[0:1, :])
                for jc in range(4):
                    with nc.allow_low_precision("f32r keeps fp32 storage"):
                        nc.vector.tensor_mul(xin[:, jc, ns], xin[:, jc, ns], r_b)

        def project(dst, xin, w_sb, aux_lhs, aux, nchunks, mm_ps, pool):
            """dst[:, jc, n] = sum_kc w_sb[:,kc,jcblk].T @ xin[:,kc,n] + aux."""
            for jc in range(4):
                js = slice(128 * jc, 128 * jc + 128)
                pmms = [mm_ps.tile([128, 512], F32, name=f"pmm{n2}",
                                   bufs=1)
                        for n2 in range(nchunks)]
                for kc in range(4):
                    for n2 in range(nchunks):
                        ns = slice(512 * n2, 512 * n2 + 512)
                        nc.tensor.matmul(pmms[n2], w_sb[:, kc, js],
                                         xin[:, kc, ns],
                                         start=(kc == 0), stop=False)
                for n2 in range(nchunks):
                    ns = slice(512 * n2, 512 * n2 + 512)
                    nc.tensor.matmul(pmms[n2], aux_lhs[:, js], aux[:, ns],
                                     start=False, stop=True)
                    nc.vector.tensor_copy(dst[:, jc, ns], pmms[n2])

        # ---- Q phase: LN + projection ----
        with tc.tile_pool(name="wq_p", bufs=1) as wq_p, \
             tc.tile_pool(name="q_sb", bufs=1) as q_sb, \
             tc.tile_pool(name="q_st", bufs=1, space="PSUM") as q_st, \
             tc.tile_pool(name="q_mm", bufs=1, space="PSUM") as q_mm:
            wq_sb = wq_p.tile([128, 4, D], BF16)
            nc.sync.dma_start(out=wq_sb, in_=wq_d)
            aq_sb = q_sb.tile([2, D], BF16)
            nc.sync.dma_start(out=aq_sb, in_=aq_d)
            qtin = q_sb.tile([128, 4, NQS], BF16)
            nc.sync.dma_start(out=qtin, in_=qt_d)
            auxq = q_sb.tile([2, NQS], BF16)
            ln_stats_prescale(qtin, 2, q_st, q_sb, auxq)
            project(qTs, qtin, wq_sb, aq_sb, auxq, 2, q_mm, q_sb)

        # ---- K phase ----
        with tc.tile_pool(name="wk_p", bufs=1) as wk_p, \
             tc.tile_pool(name="k_sb", bufs=1) as k_sb, \
             tc.tile_pool(name="k_st", bufs=1, space="PSUM") as k_st, \
             tc.tile_pool(name="k_mm", bufs=1, space="PSUM") as k_mm:
            wk_sb = wk_p.tile([128, 4, D], BF16)
            nc.sync.dma_start(out=wk_sb, in_=wk_d)
            ak_sb = k_sb.tile([2, D], BF16)
            nc.sync.dma_start(out=ak_sb, in_=ak_d)
            ktin = k_sb.tile([128, 4, NK], BF16)
            nc.sync.dma_start(out=ktin, in_=kt_d)
            auxk = k_sb.tile([2, NK], BF16)
            ln_stats_prescale(ktin, 4, k_st, k_sb, auxk)
            project(kTs, ktin, wk_sb, ak_sb, auxk, 4, k_mm, k_sb)

        # ---- V phase: plain projection into natural layout + ones col ----
        with tc.tile_pool(name="wv_p", bufs=1) as wv_p, \
             tc.tile_pool(name="v_sb", bufs=1) as v_sb, \
             tc.tile_pool(name="v_mm", bufs=1, space="PSUM") as v_mm:
            wv_sb = wv_p.tile([128, 4, D], BF16)
            nc.sync.dma_start(out=wv_sb, in_=wv_d)
            vtin = v_sb.tile([128, 4, NK], BF16)
            nc.sync.dma_start(out=vtin, in_=vt_d)
            for t in range(16):
                ts = slice(128 * t, 128 * t + 128)
                pv = v_mm.tile([128, 512], F32, name="pv", bufs=3)
                for kc in range(4):
                    nc.tensor.matmul(pv, vtin[:, kc, ts], wv_sb[:, kc, :],
                                     start=(kc == 0), stop=(kc == 3))
                nc.vector.tensor_copy(
                    vaug[:, t, :, 0:64],
                    pv.rearrange("p (h v) -> p h v", h=8))

        # ---- Attention: per head, streaming over key chunks ----
        with tc.tile_pool(name="at_sb", bufs=1) as at_sb, \
             tc.tile_pool(name="at_L", bufs=1, space="PSUM") as at_L, \
             tc.tile_pool(name="at_O", bufs=1, space="PSUM") as at_O:
            # Head pairs (2P, 2P+1) share feature chunk jc=P at partition rows
            # [0:64] / [64:128]: their K=64 L-matmuls row-tile into disjoint
            # halves of the PE array and execute concurrently. Emission is
            # software-pipelined: the L-matmuls of step p+1 enter the PE queue
            # before the O-matmuls of step p (which wait on exp(p)).
            def emit_L_pair(P, p):
                ks = slice(128 * p, 128 * p + 128)
                tiles = [at_L.tile([128, 1024], F32, name=f"psL{hh}", bufs=1)
                         for hh in range(2)]
                for c in range(2):
                    for hh in range(2):
                        rb = 64 * hh
                        nc.tensor.matmul(
                            tiles[hh][:, 512 * c : 512 * c + 512],
                            kTs[rb : rb + 64, P, ks],
                            qTs[rb : rb + 64, P, 512 * c : 512 * c + 512],
                            start=True, stop=True)
                return tiles

            for P in range(4):
                ps_o = [[at_O.tile([65, 512], F32, name=f"ps_o{hh}{c}",
                                   bufs=1) for c in range(2)]
                        for hh in range(2)]
                if P == 0:
                    psL_next = emit_L_pair(0, 0)
                for p in range(16):
                    psL = psL_next
                    exs = []
                    for hh in range(2):
                        ex = at_sb.tile([128, 1024], BF16, name=f"ex{hh}",
                                        bufs=4)
                        nc.scalar.activation(ex, psL[hh], AF.Exp,
                                             scale=1.0 / TEMP)
                        exs.append(ex)
                    if p < 15:
                        psL_next = emit_L_pair(P, p + 1)
                    elif P < 3:
                        psL_next = emit_L_pair(P + 1, 0)
                    for hh in range(2):
                        for c in range(2):
                            nc.tensor.matmul(
                                ps_o[hh][c], vaug[:, p, 2 * P + hh, :],
                                exs[hh][:, 512 * c : 512 * c + 512],
                                start=(p == 0), stop=(p == 15))
                for hh in range(2):
                    rb = 64 * hh
                    for c in range(2):
                        cs = slice(512 * c, 512 * c + 512)
                        rr = at_sb.tile([65, 512], F32R, name="rr", bufs=2)
                        with nc.allow_low_precision("f32r keeps fp32 storage"):
                            nc.vector.reciprocal(rr[64:65, :],
                                                 ps_o[hh][c][64:65, :])
                        pb = at_L.tile([64, 512], F32, name="psL0", bufs=1)
                        nc.tensor.matmul(pb, onesr[64:65, 0:64], rr[64:65, :],
                                         start=True, stop=True)
                        rbt = at_sb.tile([64, 512], F32, name="rbt", bufs=2)
                        nc.vector.tensor_copy(rbt, pb)
                        ost = at_sb.tile([64, 512], F32R, name="ost", bufs=2)
                        with nc.allow_low_precision("f32r keeps fp32 storage"):
                            nc.vector.tensor_mul(ost, ps_o[hh][c][0:64, :], rbt)
                        nc.sync.dma_start(out=oTs[rb : rb + 64, P, cs],
                                          in_=ost)

        # ---- Finale: LN fold + Wo + gelu + residual ----
        with tc.tile_pool(name="f_sb", bufs=1) as f_sb, \
             tc.tile_pool(name="f_st", bufs=1, space="PSUM") as f_st, \
             tc.tile_pool(name="f_mm", bufs=1, space="PSUM") as f_mm:
            auxo = f_sb.tile([2, NQS], F32R)
            ln_stats_prescale(oTs, 2, f_st, f_sb, auxo)
            mrbs = []
            for n2 in range(2):
                ns = slice(512 * n2, 512 * n2 + 512)
                pbm = f_mm.tile([128, 512], F32, name="pbm", bufs=2)
                nc.tensor.matmul(pbm, onesr[0:1, :], auxo[0:1, ns],
                                 start=True, stop=True)
                mrb = f_sb.tile([128, 512], F32, name=f"mrb{n2}", bufs=1)
                nc.vector.tensor_copy(mrb, pbm)
                mrbs.append(mrb)
            for jc in range(4):
                js = slice(128 * jc, 128 * jc + 128)
                for n2 in range(2):
                    ns = slice(512 * n2, 512 * n2 + 512)
                    mrb = mrbs[n2]
                    pg = f_mm.tile([128, 512], F32, name="pg", bufs=2)
                    for kc in range(4):
                        nc.tensor.matmul(pg, wo_sb[:, kc, js], oTs[:, kc, ns],
                                         start=(kc == 0), stop=False)
                    nc.tensor.matmul(pg, ao_sb[:, js], auxo[:, ns],
                                     start=False, stop=True)
                    gl = f_sb.tile([128, 512], F32, name="gl", bufs=2)
                    nc.scalar.activation(gl, pg, AF.Gelu)
                    # residual: (oTs - mr)*g + b   (oTs already prescaled by r)
                    ut = f_sb.tile([128, 512], F32, name="ut", bufs=2)
                    nc.vector.tensor_sub(ut, oTs[:, jc, ns], mrb)
                    nc.vector.tensor_scalar(
                        ut, ut, gb_sb[:, 0, jc : jc + 1],
                        gb_sb[:, 1, jc : jc + 1],
                        op0=mybir.AluOpType.mult, op1=mybir.AluOpType.add)
                    of = f_sb.tile([128, 512], F32, name="of", bufs=2)
                    nc.vector.tensor_add(of, ut, gl)
                    nc.sync.dma_start(out=out_d[:, jc, ns], in_=of)

    nc.compile()
    return nc


def _chunk_fm(x):
    """[512, N] feature-major -> [128, 4, N] (partition, chunk, col)."""
    n = x.shape[1]
    return np.ascontiguousarray(x.reshape(4, 128, n).transpose(1, 0, 2))


def _prep_inputs(Q, K, V, Wq, Wk, Wv, Wo, g, b, go, bo):
    WqT = np.ascontiguousarray((Wq * g[None, :]).T)
    WkT = np.ascontiguousarray((Wk * g[None, :]).T)
    WvT = np.ascontiguousarray(Wv.T)
    WoT = np.ascontiguousarray((Wo * go[None, :]).T)
    b16 = ml_dtypes.bfloat16
    shared = {
        f"wq_{SALT}": _chunk_fm(WqT).astype(b16),
        f"wk_{SALT}": _chunk_fm(WkT).astype(b16),
        f"wv_{SALT}": _chunk_fm(WvT).astype(b16),
        f"wo_{SALT}": _chunk_fm(WoT),
        f"aq_{SALT}": np.stack([-WqT.sum(0), Wq @ b]).astype(b16),
        f"ak_{SALT}": np.stack([-WkT.sum(0), Wk @ b]).astype(b16),
        f"ao_{SALT}": np.ascontiguousarray(np.stack([-WoT.sum(0), Wo @ bo])),
        f"gb_{SALT}": np.ascontiguousarray(
            np.stack([go.reshape(4, 128).T, bo.reshape(4, 128).T], axis=1)),
    }
    in_maps = []
    for core in range(N_CORES):
        bi, half = core // 2, core % 2
        qs = slice(half * NQS, (half + 1) * NQS)
        m = dict(shared)
        m[f"salt_{SALT}"] = np.full((1, 8 + SALT_N), LN_EPS, np.float32)
        m[f"qt_{SALT}"] = _chunk_fm(np.ascontiguousarray(Q[bi, qs, :].T)).astype(b16)
        m[f"kt_{SALT}"] = _chunk_fm(np.ascontiguousarray(K[bi].T)).astype(b16)
        m[f"vt_{SALT}"] = _chunk_fm(np.ascontiguousarray(V[bi].T)).astype(b16)
        in_maps.append(m)
    return in_maps


def kernel(Q, K, V, Wq, Wk, Wv, Wo, ln_qk_g, ln_qk_b, ln_o_g, ln_o_b,
           _trace=False):
    args = [np.asarray(a, dtype=np.float32) for a in
            (Q, K, V, Wq, Wk, Wv, Wo, ln_qk_g, ln_qk_b, ln_o_g, ln_o_b)]
    if "nc" not in _CACHE:
        _CACHE["nc"] = _build_program()
    nc = _CACHE["nc"]
    in_maps = _prep_inputs(*args)
    res = run_bass_kernel_spmd(nc, in_maps, core_ids=list(range(N_CORES)),
                               trace=_trace)
    _CACHE["last_results"] = res
    out = np.empty((B, NQ, D), dtype=np.float32)
    for core in range(N_CORES):
        bi, half = core // 2, core % 2
        o = res.results[core][f"out_{SALT}"]  # [128, 4, NQS]
        out[bi, half * NQS : (half + 1) * NQS, :] = (
            o.transpose(1, 0, 2).reshape(D, NQS).T)
    return out



# revision 6
# speedup vs baseline: 1.0749x; 1.0749x over previous
# Cross-attention kernel for Trainium2, 8 NeuronCores.
#
# Sharding: data-parallel over (batch, query-half): core = 2*b + half handles
# batch b, queries [half*1024, (half+1)*1024). No collectives.
#
# On-device layout is feature-major ("transposed"): activations live as
# [feature, token]. The host pre-transposes inputs and post-transposes the
# output. Both layernorms are folded into the projections:
#   LN(x) @ W.T = (x*r) @ W'.T - (m*r) x S + bq, with W' = (W*g).T,
#   S[j] = sum_d W'[d,j], bq = W @ b, applied via a K=2 auxiliary matmul
#   with moving rows [m*r; 1].
# Attention runs with keys on partitions (logits transposed), so softmax
# denominators come from an all-ones column appended to V (M=65 matmul
# output row 64 = sum of exp). Max-subtraction is skipped: |logits/temp| < 3.
#
# The attention inner loop staggers the two head-half (hh) pipelines so the
# scalar engine's exp stream paces the loop: per step the emission order is
# exp0(p), L0(p+1), exp1(p), O0(p), L1(p+1), O1(p). Each hh has its own psL
# PSUM tile, so L_hh(p+1) only has a WAR dependency on exp_hh(p), and the two
# chains interleave on the scalar engine with no serial L->exp->L gap.
import os
import sys
import tempfile

# The neuron compile cache keys on the jax module hash, which does not cover
# the embedded Bass program — a stale NEFF can be silently reused. Use a
# fresh cache dir per process.
os.environ["NEURON_COMPILE_CACHE_URL"] = tempfile.mkdtemp(prefix="neff_cache_")
# The axon cassette (compile record/replay) fingerprints the module interface
# but not the embedded Bass program; salt it per process so edits always
# recompile instead of replaying a stale executable.
os.environ["AXON_CASSETTE_SALT"] = f"ca-{os.getpid()}-{os.urandom(4).hex()}"

for _p in ("/opt/trn_rl_repo",):
    if os.path.isdir(_p) and _p not in sys.path:
        sys.path.insert(0, _p)

import numpy as np
import ml_dtypes
from contextlib import ExitStack

import concourse.bass as bass
import concourse.tile as tile
from concourse import bacc, mybir
from concourse.bass_utils import run_bass_kernel_spmd

F32 = mybir.dt.float32
F32R = mybir.dt.float32r
BF16 = mybir.dt.bfloat16
AF = mybir.ActivationFunctionType

B, NQ, NK, D = 4, 2048, 2048, 512
H, DH = 8, 64
NQS = NQ // 2  # queries per core
TEMP = float(np.sqrt(512.0))
LN_EPS = 1e-5
N_CORES = 8

_CACHE = {}
# Interface salt: the remote executable cache fingerprints the module
# interface but not the embedded program; bump per kernel edit to force
# recompilation.
SALT = "v11"
SALT_N = 11


def _build_program():
    nc = bacc.Bacc("TRN2", target_bir_lowering=False, debug=False)

    def din(name, shape, dt=F32R):
        return nc.dram_tensor(f"{name}_{SALT}", shape, dt,
                              kind="ExternalInput").ap()

    qt_d = din("qt", [128, 4, NQS], BF16)
    kt_d = din("kt", [128, 4, NK], BF16)
    vt_d = din("vt", [128, 4, NK], BF16)
    wq_d = din("wq", [128, 4, D], BF16)
    wk_d = din("wk", [128, 4, D], BF16)
    wv_d = din("wv", [128, 4, D], BF16)
    wo_d = din("wo", [128, 4, D])
    aq_d = din("aq", [2, D], BF16)
    ak_d = din("ak", [2, D], BF16)
    ao_d = din("ao", [2, D])
    gb_d = din("gb", [128, 2, 4], F32)
    salt_d = din("salt", [1, 8 + SALT_N], F32)
    out_d = nc.dram_tensor(f"out_{SALT}", [128, 4, NQS], F32,
                           kind="ExternalOutput").ap()

    with tile.TileContext(nc) as tc, ExitStack() as top:
        persist = top.enter_context(tc.tile_pool(name="persist", bufs=1))
        # inputs / weights
        wq_sb = persist.tile([128, 4, D], BF16)
        wk_sb = persist.tile([128, 4, D], BF16)
        wv_sb = persist.tile([128, 4, D], BF16)
        wo_sb = persist.tile([128, 4, D], F32R)
        aq_sb = persist.tile([2, D], BF16)
        ak_sb = persist.tile([2, D], BF16)
        ao_sb = persist.tile([2, D], F32R)
        gb_sb = persist.tile([128, 2, 4], F32)
        qtin = persist.tile([128, 4, NQS], BF16)
        ktin = persist.tile([128, 4, NK], BF16)
        vtin = persist.tile([128, 4, NK], BF16)
        # intermediates
        qTs = persist.tile([128, 4, NQS], BF16)       # projected q, feature-major
        kTs = persist.tile([128, 4, NK], BF16)        # projected k
        vaug = persist.tile([128, 16, 8, 65], BF16)   # v natural + ones col per head
        oTs = persist.tile([128, 4, NQS], F32R)       # normalized attention out
        # constants
        onesb = persist.tile([128, 128], BF16)
        onesr = persist.tile([128, 128], F32R)
        ones_rb = persist.tile([1, NK], BF16)
        ones_r = persist.tile([1, NK], F32R)
        eps_t = persist.tile([128, 1], F32)

        # prefetch everything, Q-phase needs first
        nc.sync.dma_start(out=wq_sb, in_=wq_d)
        nc.sync.dma_start(out=aq_sb, in_=aq_d)
        nc.sync.dma_start(out=qtin, in_=qt_d)
        nc.sync.dma_start(out=wk_sb, in_=wk_d)
        nc.sync.dma_start(out=ak_sb, in_=ak_d)
        nc.sync.dma_start(out=ktin, in_=kt_d)
        nc.sync.dma_start(out=wv_sb, in_=wv_d)
        nc.sync.dma_start(out=vtin, in_=vt_d)
        nc.sync.dma_start(out=wo_sb, in_=wo_d)
        nc.sync.dma_start(out=ao_sb, in_=ao_d)
        nc.sync.dma_start(out=gb_sb, in_=gb_d)
        nc.sync.dma_start(out=eps_t, in_=salt_d[0:1, 0:1].to_broadcast([128, 1]))
        nc.vector.memset(onesb, 1.0)
        nc.vector.memset(ones_rb, 1.0)
        nc.vector.memset(vaug[:, :, :, 64], 1.0)
        onesf = persist.tile([128, 128], F32)
        ones_rf = persist.tile([1, NK], F32)
        nc.vector.memset(onesf, 1.0)
        nc.vector.memset(ones_rf, 1.0)
        nc.vector.tensor_copy(onesr, onesf)
        nc.vector.tensor_copy(ones_r, ones_rf)

        def ln_stats_prescale(xin, nchunks, st_ps, work, aux):
            """Column LN stats of xin [128, 4, nchunks*512]; prescales xin by
            r in place; fills aux [2, nchunks*512] rows with [m*r; 1]."""
            bf = xin.dtype == BF16
            o_mm = onesb if bf else onesr
            nc.sync.dma_start(
                out=aux[1:2, :],
                in_=(ones_rb if bf else ones_r)[0:1, 0 : nchunks * 512])
            for n2 in range(nchunks):
                ns = slice(512 * n2, 512 * n2 + 512)
                ps_sum = st_ps.tile([128, 512], F32, name="ps_sum", bufs=2)
                ps_ssq = st_ps.tile([128, 512], F32, name="ps_ssq", bufs=2)
                for jc in range(4):
                    nc.tensor.matmul(ps_sum, o_mm, xin[:, jc, ns],
                                     start=(jc == 0), stop=(jc == 3))
                for jc in range(4):
                    sq = work.tile([128, 512], BF16 if bf else F32R,
                                   name="sq", bufs=3)
                    with nc.allow_low_precision("f32r keeps fp32 storage"):
                        nc.vector.tensor_mul(sq, xin[:, jc, ns],
                                             xin[:, jc, ns])
                    nc.tensor.matmul(ps_ssq, o_mm, sq,
                                     start=(jc == 0), stop=(jc == 3))
                # replicated stats rows: m = sum/512 ; var = (ssq - sum*m)/512
                m_b = work.tile([128, 512], F32, name="m_b", bufs=2)
                nc.scalar.mul(m_b, ps_sum, 1.0 / 512.0)
                t2 = work.tile([128, 512], F32, name="t2", bufs=2)
                nc.vector.tensor_mul(t2, m_b, ps_sum)
                dv = work.tile([128, 512], F32, name="dv", bufs=2)
                nc.vector.tensor_sub(dv, ps_ssq, t2)
                std = work.tile([128, 512], F32, name="std", bufs=2)
                nc.scalar.activation(std, dv, AF.Sqrt, bias=eps_t,
                                     scale=1.0 / 512.0)
                r_b = work.tile([128, 512], F32, name="r_b", bufs=2)
                nc.vector.reciprocal(r_b, std)
                mr_b = work.tile([128, 512], F32R, name="mr_b", bufs=2)
                with nc.allow_low_precision("f32r keeps fp32 storage"):
                    nc.vector.tensor_mul(mr_b, m_b, r_b)
                nc.vector.tensor_copy(aux[0:1, ns], mr_b[0:1, :])
                for jc in range(4):
                    with nc.allow_low_precision("f32r keeps fp32 storage"):
                        nc.vector.tensor_mul(xin[:, jc, ns], xin[:, jc, ns], r_b)

        def project(dst, xin, w_sb, aux_lhs, aux, nchunks, mm_ps, pool):
            """dst[:, jc, n] = sum_kc w_sb[:,kc,jcblk].T @ xin[:,kc,n] + aux."""
            for jc in range(4):
                js = slice(128 * jc, 128 * jc + 128)
                pmms = [mm_ps.tile([128, 512], F32, name=f"pmm{n2}", bufs=1)
                        for n2 in range(nchunks)]
                for kc in range(4):
                    for n2 in range(nchunks):
                        ns = slice(512 * n2, 512 * n2 + 512)
                        nc.tensor.matmul(pmms[n2], w_sb[:, kc, js],
                                         xin[:, kc, ns],
                                         start=(kc == 0), stop=False)
                for n2 in range(nchunks):
                    ns = slice(512 * n2, 512 * n2 + 512)
                    nc.tensor.matmul(pmms[n2], aux_lhs[:, js], aux[:, ns],
                                     start=False, stop=True)
                    nc.vector.tensor_copy(dst[:, jc, ns], pmms[n2])

        # ---- prologue: LN + projections, one psum pool pair shared ----
        with tc.tile_pool(name="pr_sb", bufs=1) as work, \
             tc.tile_pool(name="pr_st", bufs=1, space="PSUM") as st_ps, \
             tc.tile_pool(name="pr_mm", bufs=1, space="PSUM") as mm_ps:
            auxq = work.tile([2, NQS], BF16)
            ln_stats_prescale(qtin, 2, st_ps, work, auxq)
            project(qTs, qtin, wq_sb, aq_sb, auxq, 2, mm_ps, work)

            auxk = work.tile([2, NK], BF16)
            ln_stats_prescale(ktin, 4, st_ps, work, auxk)
            project(kTs, ktin, wk_sb, ak_sb, auxk, 4, mm_ps, work)

            # V: plain projection into natural layout + ones col
            for t in range(16):
                ts = slice(128 * t, 128 * t + 128)
                pv = mm_ps.tile([128, 512], F32, name=f"pmm{t % 4}", bufs=1)
                for kc in range(4):
                    nc.tensor.matmul(pv, vtin[:, kc, ts], wv_sb[:, kc, :],
                                     start=(kc == 0), stop=(kc == 3))
                nc.vector.tensor_copy(
                    vaug[:, t, :, 0:64],
                    pv.rearrange("p (h v) -> p h v", h=8))

        # ---- Attention: per head pair P, staggered hh chains ----
        with tc.tile_pool(name="at_sb", bufs=1) as at_sb, \
             tc.tile_pool(name="at_L", bufs=1, space="PSUM") as at_L, \
             tc.tile_pool(name="at_O", bufs=1, space="PSUM") as at_O:

            def emit_L(hh, P, p):
                """Both q-column chunks of head (2P+hh)'s logits for key
                chunk p -> [128 keys, 1024 q] PSUM tile."""
                ks = slice(128 * p, 128 * p + 128)
                rb = 64 * hh
                t = at_L.tile([128, 1024], F32, name=f"psL{hh}", bufs=1)
                for c in range(2):
                    nc.tensor.matmul(
                        t[:, 512 * c : 512 * c + 512],
                        kTs[rb : rb + 64, P, ks],
                        qTs[rb : rb + 64, P, 512 * c : 512 * c + 512],
                        start=True, stop=True)
                return t

            psL_next = [None, None]
            for P in range(4):
                ps_o = [[at_O.tile([65, 512], F32, name=f"ps_o{hh}{c}",
                                   bufs=1) for c in range(2)]
                        for hh in range(2)]
                if P == 0:
                    psL_next = [emit_L(0, 0, 0), emit_L(1, 0, 0)]
                for p in range(16):
                    psL = psL_next
                    psL_next = [None, None]
                    nP, np_ = (P, p + 1) if p < 15 else (P + 1, 0)
                    exs = [None, None]
                    for hh in range(2):
                        ex = at_sb.tile([128, 1024], BF16, name=f"ex{hh}",
                                        bufs=3)
                        nc.scalar.activation(ex, psL[hh], AF.Exp,
                                             scale=1.0 / TEMP)
                        exs[hh] = ex
                        if nP < 4:
                            psL_next[hh] = emit_L(hh, nP, np_)
                        for c in range(2):
                            nc.tensor.matmul(
                                ps_o[hh][c], vaug[:, p, 2 * P + hh, :],
                                exs[hh][:, 512 * c : 512 * c + 512],
                                start=(p == 0), stop=(p == 15))
                # epilogue: normalize by the ones-row denominators
                for hh in range(2):
                    rb = 64 * hh
                    for c in range(2):
                        cs = slice(512 * c, 512 * c + 512)
                        rr = at_sb.tile([65, 512], F32, name="rr", bufs=2)
                        nc.vector.reciprocal(rr[64:65, :],
                                             ps_o[hh][c][64:65, :])
                        rrr = at_sb.tile([65, 512], F32R, name="rrr", bufs=2)
                        nc.vector.tensor_copy(rrr[64:65, :], rr[64:65, :])
                        pb = at_L.tile([64, 512], F32, name=f"psL{hh}",
                                       bufs=1)
                        nc.tensor.matmul(pb, onesr[64:65, 0:64],
                                         rrr[64:65, :], start=True, stop=True)
                        rbt = at_sb.tile([64, 512], F32, name="rbt", bufs=2)
                        nc.vector.tensor_copy(rbt, pb)
                        ost = at_sb.tile([64, 512], F32R, name="ost", bufs=2)
                        with nc.allow_low_precision("f32r keeps fp32 storage"):
                            nc.vector.tensor_mul(ost, ps_o[hh][c][0:64, :], rbt)
                        nc.sync.dma_start(out=oTs[rb : rb + 64, P, cs],
                                          in_=ost)

        # ---- Finale: LN fold + Wo + gelu + residual ----
        with tc.tile_pool(name="f_sb", bufs=1) as f_sb, \
             tc.tile_pool(name="f_st", bufs=1, space="PSUM") as f_st, \
             tc.tile_pool(name="f_mm", bufs=1, space="PSUM") as f_mm:
            auxo = f_sb.tile([2, NQS], F32R)
            ln_stats_prescale(oTs, 2, f_st, f_sb, auxo)
            mrbs = []
            for n2 in range(2):
                ns = slice(512 * n2, 512 * n2 + 512)
                pbm = f_mm.tile([128, 512], F32, name="pbm", bufs=2)
                nc.tensor.matmul(pbm, onesr[0:1, :], auxo[0:1, ns],
                                 start=True, stop=True)
                mrb = f_sb.tile([128, 512], F32, name=f"mrb{n2}", bufs=1)
                nc.vector.tensor_copy(mrb, pbm)
                mrbs.append(mrb)
            for jc in range(4):
                js = slice(128 * jc, 128 * jc + 128)
                for n2 in range(2):
                    ns = slice(512 * n2, 512 * n2 + 512)
                    mrb = mrbs[n2]
                    pg = f_mm.tile([128, 512], F32, name="pg", bufs=2)
                    for kc in range(4):
                        nc.tensor.matmul(pg, wo_sb[:, kc, js], oTs[:, kc, ns],
                                         start=(kc == 0), stop=False)
                    nc.tensor.matmul(pg, ao_sb[:, js], auxo[:, ns],
                                     start=False, stop=True)
                    gl = f_sb.tile([128, 512], F32, name="gl", bufs=2)
                    nc.scalar.activation(gl, pg, AF.Gelu)
                    # residual: (oTs - mr)*g + b   (oTs already prescaled by r)
                    ut = f_sb.tile([128, 512], F32, name="ut", bufs=2)
                    nc.vector.tensor_sub(ut, oTs[:, jc, ns], mrb)
                    nc.vector.tensor_scalar(
                        ut, ut, gb_sb[:, 0, jc : jc + 1],
                        gb_sb[:, 1, jc : jc + 1],
                        op0=mybir.AluOpType.mult, op1=mybir.AluOpType.add)
                    of = f_sb.tile([128, 512], F32, name="of", bufs=2)
                    nc.vector.tensor_add(of, ut, gl)
                    nc.sync.dma_start(out=out_d[:, jc, ns], in_=of)

    nc.compile()
    return nc


def _chunk_fm(x):
    """[512, N] feature-major -> [128, 4, N] (partition, chunk, col)."""
    n = x.shape[1]
    return np.ascontiguousarray(x.reshape(4, 128, n).transpose(1, 0, 2))


def _prep_inputs(Q, K, V, Wq, Wk, Wv, Wo, g, b, go, bo):
    WqT = np.ascontiguousarray((Wq * g[None, :]).T)
    WkT = np.ascontiguousarray((Wk * g[None, :]).T)
    WvT = np.ascontiguousarray(Wv.T)
    WoT = np.ascontiguousarray((Wo * go[None, :]).T)
    b16 = ml_dtypes.bfloat16
    shared = {
        f"wq_{SALT}": _chunk_fm(WqT).astype(b16),
        f"wk_{SALT}": _chunk_fm(WkT).astype(b16),
        f"wv_{SALT}": _chunk_fm(WvT).astype(b16),
        f"wo_{SALT}": _chunk_fm(WoT),
        f"aq_{SALT}": np.stack([-WqT.sum(0), Wq @ b]).astype(b16),
        f"ak_{SALT}": np.stack([-WkT.sum(0), Wk @ b]).astype(b16),
        f"ao_{SALT}": np.ascontiguousarray(np.stack([-WoT.sum(0), Wo @ bo])),
        f"gb_{SALT}": np.ascontiguousarray(
            np.stack([go.reshape(4, 128).T, bo.reshape(4, 128).T], axis=1)),
    }
    in_maps = []
    for core in range(N_CORES):
        bi, half = core // 2, core % 2
        qs = slice(half * NQS, (half + 1) * NQS)
        m = dict(shared)
        m[f"salt_{SALT}"] = np.full((1, 8 + SALT_N), LN_EPS, np.float32)
        m[f"qt_{SALT}"] = _chunk_fm(np.ascontiguousarray(Q[bi, qs, :].T)).astype(b16)
        m[f"kt_{SALT}"] = _chunk_fm(np.ascontiguousarray(K[bi].T)).astype(b16)
        m[f"vt_{SALT}"] = _chunk_fm(np.ascontiguousarray(V[bi].T)).astype(b16)
        in_maps.append(m)
    return in_maps


def kernel(Q, K, V, Wq, Wk, Wv, Wo, ln_qk_g, ln_qk_b, ln_o_g, ln_o_b,
           _trace=False):
    args = [np.asarray(a, dtype=np.float32) for a in
            (Q, K, V, Wq, Wk, Wv, Wo, ln_qk_g, ln_qk_b, ln_o_g, ln_o_b)]
    if "nc" not in _CACHE:
        _CACHE["nc"] = _build_program()
    nc = _CACHE["nc"]
    in_maps = _prep_inputs(*args)
    res = run_bass_kernel_spmd(nc, in_maps, core_ids=list(range(N_CORES)),
                               trace=_trace)
    _CACHE["last_results"] = res
    out = np.empty((B, NQ, D), dtype=np.float32)
    for core in range(N_CORES):
        bi, half = core // 2, core % 2
        o = res.results[core][f"out_{SALT}"]  # [128, 4, NQS]
        out[bi, half * NQS : (half + 1) * NQS, :] = (
            o.transpose(1, 0, 2).reshape(D, NQS).T)
    return out


# revision 9
# speedup vs baseline: 1.0796x; 1.0044x over previous
# Cross-attention kernel for Trainium2, 8 NeuronCores.
#
# Sharding: data-parallel over (batch, query-half): core = 2*b + half handles
# batch b, queries [half*1024, (half+1)*1024). No collectives.
#
# On-device layout is feature-major ("transposed"): activations live as
# [feature, token]. The host pre-transposes inputs and post-transposes the
# output. Both layernorms are folded into the projections:
#   LN(x) @ W.T = (x*r) @ W'.T - (m*r) x S + bq, with W' = (W*g).T,
#   S[j] = sum_d W'[d,j], bq = W @ b, applied via a K=2 auxiliary matmul
#   with moving rows [m*r; 1].
# Attention runs with keys on partitions (logits transposed), so softmax
# denominators come from an all-ones column appended to V (M=65 matmul
# output row 64 = sum of exp). Max-subtraction is skipped: |logits/temp| < 3.
#
# The attention inner loop staggers the two head-half (hh) pipelines so the
# scalar engine's exp stream paces the loop: per step the emission order is
# exp0(p), L0(p+1), exp1(p), O0(p), L1(p+1), O1(p). Each hh has its own psL
# PSUM tile, so L_hh(p+1) only has a WAR dependency on exp_hh(p), and the two
# chains interleave on the scalar engine with no serial L->exp->L gap.
import os
import sys
import tempfile

# The neuron compile cache keys on the jax module hash, which does not cover
# the embedded Bass program — a stale NEFF can be silently reused. Use a
# fresh cache dir per process.
os.environ["NEURON_COMPILE_CACHE_URL"] = tempfile.mkdtemp(prefix="neff_cache_")
# The axon cassette (compile record/replay) fingerprints the module interface
# but not the embedded Bass program; salt it per process so edits always
# recompile instead of replaying a stale executable.
os.environ["AXON_CASSETTE_SALT"] = f"ca-{os.getpid()}-{os.urandom(4).hex()}"

for _p in ("/opt/trn_rl_repo",):
    if os.path.isdir(_p) and _p not in sys.path:
        sys.path.insert(0, _p)

import numpy as np
import ml_dtypes
from contextlib import ExitStack

import concourse.bass as bass
import concourse.tile as tile
from concourse import bacc, mybir
from concourse.bass_utils import run_bass_kernel_spmd

F32 = mybir.dt.float32
F32R = mybir.dt.float32r
BF16 = mybir.dt.bfloat16
AF = mybir.ActivationFunctionType

B, NQ, NK, D = 4, 2048, 2048, 512
H, DH = 8, 64
NQS = NQ // 2  # queries per core
TEMP = float(np.sqrt(512.0))
LN_EPS = 1e-5
N_CORES = 8

_CACHE = {}
# Interface salt: the remote executable cache fingerprints the module
# interface but not the embedded program; bump per kernel edit to force
# recompilation.
SALT = "v12"
SALT_N = 12


def _build_program():
    nc = bacc.Bacc("TRN2", target_bir_lowering=False, debug=False)

    def din(name, shape, dt=F32R):
        return nc.dram_tensor(f"{name}_{SALT}", shape, dt,
                              kind="ExternalInput").ap()

    qt_d = din("qt", [128, 4, NQS], BF16)
    kt_d = din("kt", [128, 4, NK], BF16)
    vt_d = din("vt", [128, 4, NK], BF16)
    wq_d = din("wq", [128, 4, D], BF16)
    wk_d = din("wk", [128, 4, D], BF16)
    wv_d = din("wv", [128, 4, D], BF16)
    wo_d = din("wo", [128, 4, D])
    aq_d = din("aq", [2, D], BF16)
    ak_d = din("ak", [2, D], BF16)
    ao_d = din("ao", [2, D])
    gb_d = din("gb", [128, 2, 4], F32)
    salt_d = din("salt", [1, 8 + SALT_N], F32)
    out_d = nc.dram_tensor(f"out_{SALT}", [128, 4, NQS], F32,
                           kind="ExternalOutput").ap()

    with tile.TileContext(nc) as tc, ExitStack() as top:
        persist = top.enter_context(tc.tile_pool(name="persist", bufs=1))
        # inputs / weights
        wq_sb = persist.tile([128, 4, D], BF16)
        wk_sb = persist.tile([128, 4, D], BF16)
        wv_sb = persist.tile([128, 4, D], BF16)
        wo_sb = persist.tile([128, 4, D], F32R)
        aq_sb = persist.tile([2, D], BF16)
        ak_sb = persist.tile([2, D], BF16)
        ao_sb = persist.tile([2, D], F32R)
        gb_sb = persist.tile([128, 2, 4], F32)
        qtin = persist.tile([128, 4, NQS], BF16)
        ktin = persist.tile([128, 4, NK], BF16)
        vtin = persist.tile([128, 4, NK], BF16)
        # intermediates
        qTs = persist.tile([128, 4, NQS], BF16)       # projected q, feature-major
        kTs = persist.tile([128, 4, NK], BF16)        # projected k
        vaug = persist.tile([128, 16, 8, 65], BF16)   # v natural + ones col per head
        oTs = persist.tile([128, 4, NQS], F32R)       # normalized attention out
        # constants
        onesb = persist.tile([128, 128], BF16)
        onesr = persist.tile([128, 128], F32R)
        ones_rb = persist.tile([1, NK], BF16)
        ones_r = persist.tile([1, NK], F32R)
        eps_t = persist.tile([128, 1], F32)

        # prefetch everything, Q-phase needs first
        nc.sync.dma_start(out=wq_sb, in_=wq_d)
        nc.sync.dma_start(out=aq_sb, in_=aq_d)
        nc.sync.dma_start(out=qtin, in_=qt_d)
        nc.sync.dma_start(out=wk_sb, in_=wk_d)
        nc.sync.dma_start(out=ak_sb, in_=ak_d)
        nc.sync.dma_start(out=ktin, in_=kt_d)
        nc.sync.dma_start(out=wv_sb, in_=wv_d)
        nc.sync.dma_start(out=vtin, in_=vt_d)
        nc.sync.dma_start(out=wo_sb, in_=wo_d)
        nc.sync.dma_start(out=ao_sb, in_=ao_d)
        nc.sync.dma_start(out=gb_sb, in_=gb_d)
        nc.sync.dma_start(out=eps_t, in_=salt_d[0:1, 0:1].to_broadcast([128, 1]))
        nc.vector.memset(onesb, 1.0)
        nc.vector.memset(ones_rb, 1.0)
        nc.vector.memset(vaug[:, :, :, 64], 1.0)
        onesf = persist.tile([128, 128], F32)
        ones_rf = persist.tile([1, NK], F32)
        nc.vector.memset(onesf, 1.0)
        nc.vector.memset(ones_rf, 1.0)
        nc.vector.tensor_copy(onesr, onesf)
        nc.vector.tensor_copy(ones_r, ones_rf)

        def ln_stats_prescale(xin, nchunks, st_ps, work, aux):
            """Column LN stats of xin [128, 4, nchunks*512]; prescales xin by
            r in place; fills aux [2, nchunks*512] rows with [m*r; 1]."""
            bf = xin.dtype == BF16
            o_mm = onesb if bf else onesr
            nc.sync.dma_start(
                out=aux[1:2, :],
                in_=(ones_rb if bf else ones_r)[0:1, 0 : nchunks * 512])
            for n2 in range(nchunks):
                ns = slice(512 * n2, 512 * n2 + 512)
                ps_sum = st_ps.tile([128, 512], F32, name="ps_sum", bufs=2)
                ps_ssq = st_ps.tile([128, 512], F32, name="ps_ssq", bufs=2)
                for jc in range(4):
                    nc.tensor.matmul(ps_sum, o_mm, xin[:, jc, ns],
                                     start=(jc == 0), stop=(jc == 3))
                for jc in range(4):
                    sq = work.tile([128, 512], BF16 if bf else F32R,
                                   name="sq", bufs=3)
                    with nc.allow_low_precision("f32r keeps fp32 storage"):
                        nc.vector.tensor_mul(sq, xin[:, jc, ns],
                                             xin[:, jc, ns])
                    nc.tensor.matmul(ps_ssq, o_mm, sq,
                                     start=(jc == 0), stop=(jc == 3))
                # replicated stats rows: m = sum/512 ; var = (ssq - sum*m)/512
                m_b = work.tile([128, 512], F32, name="m_b", bufs=2)
                nc.scalar.mul(m_b, ps_sum, 1.0 / 512.0)
                t2 = work.tile([128, 512], F32, name="t2", bufs=2)
                nc.vector.tensor_mul(t2, m_b, ps_sum)
                dv = work.tile([128, 512], F32, name="dv", bufs=2)
                nc.vector.tensor_sub(dv, ps_ssq, t2)
                std = work.tile([128, 512], F32, name="std", bufs=2)
                nc.scalar.activation(std, dv, AF.Sqrt, bias=eps_t,
                                     scale=1.0 / 512.0)
                r_b = work.tile([128, 512], F32, name="r_b", bufs=2)
                nc.vector.reciprocal(r_b, std)
                mr_b = work.tile([128, 512], F32R, name="mr_b", bufs=2)
                with nc.allow_low_precision("f32r keeps fp32 storage"):
                    nc.vector.tensor_mul(mr_b, m_b, r_b)
                nc.vector.tensor_copy(aux[0:1, ns], mr_b[0:1, :])
                for jc in range(4):
                    with nc.allow_low_precision("f32r keeps fp32 storage"):
                        nc.vector.tensor_mul(xin[:, jc, ns], xin[:, jc, ns], r_b)

        def project(dst, xin, w_sb, aux_lhs, aux, nchunks, mm_ps, pool):
            """dst[:, jc, n] = sum_kc w_sb[:,kc,jcblk].T @ xin[:,kc,n] + aux."""
            for jc in range(4):
                js = slice(128 * jc, 128 * jc + 128)
                pmms = [mm_ps.tile([128, 512], F32, name=f"pmm{n2}", bufs=1)
                        for n2 in range(nchunks)]
                for kc in range(4):
                    for n2 in range(nchunks):
                        ns = slice(512 * n2, 512 * n2 + 512)
                        nc.tensor.matmul(pmms[n2], w_sb[:, kc, js],
                                         xin[:, kc, ns],
                                         start=(kc == 0), stop=False)
                for n2 in range(nchunks):
                    ns = slice(512 * n2, 512 * n2 + 512)
                    nc.tensor.matmul(pmms[n2], aux_lhs[:, js], aux[:, ns],
                                     start=False, stop=True)
                    nc.vector.tensor_copy(dst[:, jc, ns], pmms[n2])

        # ---- prologue: LN + projections, one psum pool pair shared ----
        with tc.tile_pool(name="pr_sb", bufs=1) as work, \
             tc.tile_pool(name="pr_st", bufs=1, space="PSUM") as st_ps, \
             tc.tile_pool(name="pr_mm", bufs=1, space="PSUM") as mm_ps:
            auxq = work.tile([2, NQS], BF16)
            ln_stats_prescale(qtin, 2, st_ps, work, auxq)
            project(qTs, qtin, wq_sb, aq_sb, auxq, 2, mm_ps, work)

            auxk = work.tile([2, NK], BF16)
            ln_stats_prescale(ktin, 4, st_ps, work, auxk)
            project(kTs, ktin, wk_sb, ak_sb, auxk, 4, mm_ps, work)

            # V: plain projection into natural layout + ones col
            for t in range(16):
                ts = slice(128 * t, 128 * t + 128)
                pv = mm_ps.tile([128, 512], F32, name=f"pmm{t % 4}", bufs=1)
                for kc in range(4):
                    nc.tensor.matmul(pv, vtin[:, kc, ts], wv_sb[:, kc, :],
                                     start=(kc == 0), stop=(kc == 3))
                nc.vector.tensor_copy(
                    vaug[:, t, :, 0:64],
                    pv.rearrange("p (h v) -> p h v", h=8))

        # ---- Attention: per head pair P, staggered hh chains ----
        with tc.tile_pool(name="at_sb", bufs=1) as at_sb, \
             tc.tile_pool(name="at_L", bufs=1, space="PSUM") as at_L, \
             tc.tile_pool(name="at_O", bufs=1, space="PSUM") as at_O:

            def emit_L(hh, P, p):
                """Both q-column chunks of head (2P+hh)'s logits for key
                chunk p -> [128 keys, 1024 q] PSUM tile."""
                ks = slice(128 * p, 128 * p + 128)
                rb = 64 * hh
                t = at_L.tile([128, 1024], F32, name=f"psL{hh}", bufs=1)
                for c in range(2):
                    nc.tensor.matmul(
                        t[:, 512 * c : 512 * c + 512],
                        kTs[rb : rb + 64, P, ks],
                        qTs[rb : rb + 64, P, 512 * c : 512 * c + 512],
                        start=True, stop=True)
                return t

            psL_next = [None, None]
            DELAY = 4  # steps between exp(p) and its O-matmuls
            for P in range(4):
                ps_o = [[at_O.tile([65, 512], F32, name=f"ps_o{hh}{c}",
                                   bufs=1) for c in range(2)]
                        for hh in range(2)]
                if P == 0:
                    psL_next = [emit_L(0, 0, 0), emit_L(1, 0, 0)]
                exq = {}

                def emit_O(p):
                    for hh in range(2):
                        ex = exq.pop((p, hh))
                        for c in range(2):
                            nc.tensor.matmul(
                                ps_o[hh][c], vaug[:, p, 2 * P + hh, :],
                                ex[:, 512 * c : 512 * c + 512],
                                start=(p == 0), stop=(p == 15))

                for p in range(16 + DELAY):
                    # delayed O's first: their exp deps completed DELAY
                    # steps ago, so the PE dispatches them without waiting
                    # and the stream stays continuous (full p-state).
                    if p >= DELAY:
                        emit_O(p - DELAY)
                    if p < 16:
                        psL = psL_next
                        psL_next = [None, None]
                        nP, np_ = (P, p + 1) if p < 15 else (P + 1, 0)
                        for hh in range(2):
                            ex = at_sb.tile([128, 1024], BF16,
                                            name=f"ex{hh}", bufs=6)
                            nc.scalar.activation(ex, psL[hh], AF.Exp,
                                                 scale=1.0 / TEMP)
                            exq[(p, hh)] = ex
                            if nP < 4:
                                psL_next[hh] = emit_L(hh, nP, np_)
                # epilogue: normalize by the ones-row denominators
                for hh in range(2):
                    rb = 64 * hh
                    for c in range(2):
                        cs = slice(512 * c, 512 * c + 512)
                        rr = at_sb.tile([65, 512], F32, name="rr", bufs=2)
                        nc.vector.reciprocal(rr[64:65, :],
                                             ps_o[hh][c][64:65, :])
                        rrr = at_sb.tile([65, 512], F32R, name="rrr", bufs=2)
                        nc.vector.tensor_copy(rrr[64:65, :], rr[64:65, :])
                        pb = at_L.tile([64, 512], F32, name=f"psL{hh}",
                                       bufs=1)
                        nc.tensor.matmul(pb, onesr[64:65, 0:64],
                                         rrr[64:65, :], start=True, stop=True)
                        rbt = at_sb.tile([64, 512], F32, name="rbt", bufs=2)
                        nc.vector.tensor_copy(rbt, pb)
                        ost = at_sb.tile([64, 512], F32R, name="ost", bufs=2)
                        with nc.allow_low_precision("f32r keeps fp32 storage"):
                            nc.vector.tensor_mul(ost, ps_o[hh][c][0:64, :], rbt)
                        nc.sync.dma_start(out=oTs[rb : rb + 64, P, cs],
                                          in_=ost)

        # ---- Finale: LN fold + Wo + gelu + residual ----
        with tc.tile_pool(name="f_sb", bufs=1) as f_sb, \
             tc.tile_pool(name="f_st", bufs=1, space="PSUM") as f_st, \
             tc.tile_pool(name="f_mm", bufs=1, space="PSUM") as f_mm:
            auxo = f_sb.tile([2, NQS], F32R)
            ln_stats_prescale(oTs, 2, f_st, f_sb, auxo)
            mrbs = []
            for n2 in range(2):
                ns = slice(512 * n2, 512 * n2 + 512)
                pbm = f_mm.tile([128, 512], F32, name="pbm", bufs=2)
                nc.tensor.matmul(pbm, onesr[0:1, :], auxo[0:1, ns],
                                 start=True, stop=True)
                mrb = f_sb.tile([128, 512], F32, name=f"mrb{n2}", bufs=1)
                nc.vector.tensor_copy(mrb, pbm)
                mrbs.append(mrb)
            for jc in range(4):
                js = slice(128 * jc, 128 * jc + 128)
                for n2 in range(2):
                    ns = slice(512 * n2, 512 * n2 + 512)
                    mrb = mrbs[n2]
                    pg = f_mm.tile([128, 512], F32, name="pg", bufs=2)
                    for kc in range(4):
                        nc.tensor.matmul(pg, wo_sb[:, kc, js], oTs[:, kc, ns],
                                         start=(kc == 0), stop=False)
                    nc.tensor.matmul(pg, ao_sb[:, js], auxo[:, ns],
                                     start=False, stop=True)
                    gl = f_sb.tile([128, 512], F32, name="gl", bufs=2)
                    nc.scalar.activation(gl, pg, AF.Gelu)
                    # residual: (oTs - mr)*g + b   (oTs already prescaled by r)
                    ut = f_sb.tile([128, 512], F32, name="ut", bufs=2)
                    nc.vector.tensor_sub(ut, oTs[:, jc, ns], mrb)
                    nc.vector.tensor_scalar(
                        ut, ut, gb_sb[:, 0, jc : jc + 1],
                        gb_sb[:, 1, jc : jc + 1],
                        op0=mybir.AluOpType.mult, op1=mybir.AluOpType.add)
                    of = f_sb.tile([128, 512], F32, name="of", bufs=2)
                    nc.vector.tensor_add(of, ut, gl)
                    nc.sync.dma_start(out=out_d[:, jc, ns], in_=of)

    nc.compile()
    return nc


def _chunk_fm(x):
    """[512, N] feature-major -> [128, 4, N] (partition, chunk, col)."""
    n = x.shape[1]
    return np.ascontiguousarray(x.reshape(4, 128, n).transpose(1, 0, 2))


def _prep_inputs(Q, K, V, Wq, Wk, Wv, Wo, g, b, go, bo):
    WqT = np.ascontiguousarray((Wq * g[None, :]).T)
    WkT = np.ascontiguousarray((Wk * g[None, :]).T)
    WvT = np.ascontiguousarray(Wv.T)
    WoT = np.ascontiguousarray((Wo * go[None, :]).T)
    b16 = ml_dtypes.bfloat16
    shared = {
        f"wq_{SALT}": _chunk_fm(WqT).astype(b16),
        f"wk_{SALT}": _chunk_fm(WkT).astype(b16),
        f"wv_{SALT}": _chunk_fm(WvT).astype(b16),
        f"wo_{SALT}": _chunk_fm(WoT),
        f"aq_{SALT}": np.stack([-WqT.sum(0), Wq @ b]).astype(b16),
        f"ak_{SALT}": np.stack([-WkT.sum(0), Wk @ b]).astype(b16),
        f"ao_{SALT}": np.ascontiguousarray(np.stack([-WoT.sum(0), Wo @ bo])),
        f"gb_{SALT}": np.ascontiguousarray(
            np.stack([go.reshape(4, 128).T, bo.reshape(4, 128).T], axis=1)),
    }
    in_maps = []
    for core in range(N_CORES):
        bi, half = core // 2, core % 2
        qs = slice(half * NQS, (half + 1) * NQS)
        m = dict(shared)
        m[f"salt_{SALT}"] = np.full((1, 8 + SALT_N), LN_EPS, np.float32)
        m[f"qt_{SALT}"] = _chunk_fm(np.ascontiguousarray(Q[bi, qs, :].T)).astype(b16)
        m[f"kt_{SALT}"] = _chunk_fm(np.ascontiguousarray(K[bi].T)).astype(b16)
        m[f"vt_{SALT}"] = _chunk_fm(np.ascontiguousarray(V[bi].T)).astype(b16)
        in_maps.append(m)
    return in_maps


def kernel(Q, K, V, Wq, Wk, Wv, Wo, ln_qk_g, ln_qk_b, ln_o_g, ln_o_b,
           _trace=False):
    args = [np.asarray(a, dtype=np.float32) for a in
            (Q, K, V, Wq, Wk, Wv, Wo, ln_qk_g, ln_qk_b, ln_o_g, ln_o_b)]
    if "nc" not in _CACHE:
        _CACHE["nc"] = _build_program()
    nc = _CACHE["nc"]
    in_maps = _prep_inputs(*args)
    res = run_bass_kernel_spmd(nc, in_maps, core_ids=list(range(N_CORES)),
                               trace=_trace)
    _CACHE["last_results"] = res
    out = np.empty((B, NQ, D), dtype=np.float32)
    for core in range(N_CORES):
        bi, half = core // 2, core % 2
        o = res.results[core][f"out_{SALT}"]  # [128, 4, NQS]
        out[bi, half * NQS : (half + 1) * NQS, :] = (
            o.transpose(1, 0, 2).reshape(D, NQS).T)
    return out
